# revision 5
# baseline (speedup 1.0000x reference)
"""Trainium2 Bass/Tile kernel for a dilated-attention encoder layer.

Reference computation (per full input x of shape (4, 2048, 1024)):
  - split sequence into DILATION=2 interleaved groups -> 8 independent
    (batch, group) attention problems of shape (1024, 1024)
  - q/k/v projections, 16-head attention (dh=64), output projection,
    residual + LN, FFN(4096) with relu, residual + LN.

Sharding: pure data-parallel. Core g handles (b, j) = (g//2, g%2),
i.e. rows x[b, j::2, :].  Each core receives its own (1024, 1024) slice
plus a full copy of all weights, and produces a (1024, 1024) output
slice.  No collectives.

On-core dataflow (L = 1024 positions, D = 1024 features):
  A: load x_seq f32, PE-transpose -> x_T (bf16, feature-major)
  B: v = x @ Wv  (seq-major, with an extra all-ones column per head ->
     attention denominators come for free out of the ctx matmul)
  C: per head-pair: q_T/k_T = W^T @ x_T (feature-major);
     per head: scoresT[ki,qi] = k @ q^T, exp on ACT (no max subtraction:
     fp32 exp of |s|<~40 is safe), ctx^T = v_aug^T-style matmul with the
     ones column producing the denominator row; normalize via
     gpsimd partition_broadcast of 1/denom + DVE multiply.
  D: atted = ctx @ Wo (+x +bo computed in PSUM), LN1 stats straight from
     PSUM, affine, -> final (seq bf16) and PE-transpose -> final_T
  E/F: FFN in two L-halves: h_T = relu(W1^T @ final_T + b1) (bf16),
     out2 = h @ W2 accumulated in all 8 PSUM banks, +final +b2, LN2,
     affine, DMA out (f32).

All matmuls run in bf16 (measured end-to-end rel err ~2e-3 vs the f32
reference); everything else (PSUM accumulation, softmax, LN) is f32.
"""

import numpy as np

B_FULL, S_FULL, D = 4, 2048, 1024
DIL = 2
L = S_FULL // DIL  # 1024 positions per core
NH, DH = 16, 64
HID = 4096
N_CORES = 8
LN_EPS = 1e-5

WEIGHT_KEYS = [
    "Wq", "bq", "Wk", "bk", "Wv", "bv", "Wo", "bo",
    "ln1_g", "ln1_b", "W1", "b1", "W2", "b2", "ln2_g", "ln2_b",
]

_CACHE = {}


def _build_nc(repeat=1):
    from contextlib import ExitStack

    import concourse.bass as bass
    import concourse.tile as tile
    from concourse import bacc, mybir
    from concourse.masks import make_identity

    F32 = mybir.dt.float32
    BF16 = mybir.dt.bfloat16
    AF = mybir.ActivationFunctionType
    ALU = mybir.AluOpType
    P = 128
    LT = L // P      # 8 L-tiles
    DT = D // P      # 8 D-tiles
    HT = HID // P    # 32 hidden tiles

    nc = bacc.Bacc("TRN2", target_bir_lowering=False, debug=False)

    xg = nc.dram_tensor("xg", [L, D], F32, kind="ExternalInput").ap()
    w = {}
    for k in WEIGHT_KEYS:
        if k in ("W1",):
            shp = [D, HID]
        elif k in ("W2",):
            shp = [HID, D]
        elif k.startswith("W"):
            shp = [D, D]
        elif k in ("b1",):
            shp = [HID]
        else:
            shp = [D]
        w[k] = nc.dram_tensor(k, shp, F32, kind="ExternalInput").ap()
    out = nc.dram_tensor("out", [L, D], F32, kind="ExternalOutput").ap()

    def bcast_ap(vec_ap, parts=P):
        # partition-stride-0 DMA source: replicate a [N] dram vector across partitions
        return bass.AP(
            tensor=vec_ap.tensor, offset=vec_ap.offset,
            ap=[[0, parts]] + list(vec_ap.ap),
        )

    with tile.TileContext(nc) as tc, ExitStack() as ctx:
        # ---------------- pools ----------------
        sb = ctx.enter_context(tc.tile_pool(name="sb", bufs=2))
        ps = ctx.enter_context(tc.tile_pool(name="ps", bufs=4, space="PSUM"))
        if repeat > 1:  # timing-only variant: loop the whole body on-device
            ctx.enter_context(tc.For_i(0, repeat, 1))

        def spool(shape, dtype, tag, bufs, name):
            return sb.tile(shape, dtype, tag=tag, bufs=bufs, name=name)

        def ps1(name):
            return ps.tile([P, 512], F32, tag="ps1", bufs=4, name=name)

        def ps2(name, dtype=F32, shape=(P, 1024)):
            return ps.tile(list(shape), dtype, tag="ps2", bufs=2,
                           padded_shape=[P, 4096 // mybir.dt.size(dtype)],
                           name=name)

        # ---------------- constants ----------------
        ident_f32 = spool([P, P], F32, "idf", 1, "ident_f32")
        make_identity(nc, ident_f32)
        ident_bf = spool([P, P], BF16, "idb", 1, "ident_bf")
        make_identity(nc, ident_bf)
        eps_t = spool([P, 1], F32, "eps", 1, "eps_t")
        nc.vector.memset(eps_t, LN_EPS)

        # per-partition bias columns: [vec] viewed as [128, n]
        bq_col = spool([P, DT], F32, "bqc", 1, "bq_col")
        nc.sync.dma_start(out=bq_col, in_=w["bq"].rearrange("(i p) -> p i", p=P))
        bk_col = spool([P, DT], F32, "bkc", 1, "bk_col")
        nc.sync.dma_start(out=bk_col, in_=w["bk"].rearrange("(i p) -> p i", p=P))
        b1_col = spool([P, HT], F32, "b1c", 1, "b1_col")
        nc.sync.dma_start(out=b1_col, in_=w["b1"].rearrange("(i p) -> p i", p=P))

        # broadcast rows [128, 1024] (free-dim vectors replicated on partitions)
        def load_bcast(vec, name):
            t = spool([P, D], F32, "bcast", 3, name)
            nc.gpsimd.dma_start(out=t, in_=bcast_ap(vec))
            return t

        bv_b = load_bcast(w["bv"], "bv_b")

        # ---------------- A: load x, transpose to x_T (bf16) ----------------
        x_T = [spool([P, L], BF16, "xT", DT, f"x_T{j}") for j in range(DT)]
        for m in range(LT):
            x_seq = spool([P, D], F32, "xf32", 2, f"x_seq{m}")
            nc.sync.dma_start(out=x_seq, in_=xg[m * P:(m + 1) * P, :])
            for j in range(DT):
                tr = ps1(f"trA_{m}_{j}")
                nc.tensor.transpose(tr[0:P, 0:P], x_seq[:, j * P:(j + 1) * P],
                                    ident_f32)
                nc.vector.tensor_copy(out=x_T[j][:, m * P:(m + 1) * P],
                                      in_=tr[0:P, 0:P])

        # ---------------- B: v projection (seq-major, 65-wide per head) ----
        # v_aug[m] viewed [128, 16, 65]: cols 0..63 = v head h, col 64 = 1.0
        wv_sb = []
        for kt in range(DT):
            stg = spool([P, D], F32, "wstage", 2, f"wv_stg{kt}")
            nc.sync.dma_start(
                out=stg, in_=w["Wv"].rearrange("(a p) n -> p a n", p=P)[:, kt, :])
            wv = spool([P, D], BF16, "wtile", 8, f"wv_sb{kt}")
            nc.gpsimd.tensor_copy(out=wv, in_=stg)
            wv_sb.append(wv)
        v_aug = []
        for m in range(LT):
            va = spool([P, NH, DH + 1], BF16, "vaug", LT, f"v_aug{m}")
            nc.gpsimd.memset(va[:, :, DH:DH + 1], 1.0)
            for half in range(2):
                pv = ps1(f"psv_{m}_{half}")
                for kt in range(DT):
                    nc.tensor.matmul(
                        pv[:, 0:512],
                        lhsT=x_T[kt][:, m * P:(m + 1) * P],
                        rhs=wv_sb[kt][:, half * 512:(half + 1) * 512],
                        start=(kt == 0), stop=(kt == DT - 1))
                nc.vector.tensor_add(
                    out=va[:, half * 8:(half + 1) * 8, 0:DH],
                    in0=pv.rearrange("p (h c) -> p h c", c=DH),
                    in1=bv_b[:, half * 512:(half + 1) * 512].rearrange(
                        "p (h c) -> p h c", c=DH))
            v_aug.append(va)

        # ---------------- C: per head-pair q/k projection + attention ------
        ctxT = [spool([P, L], BF16, "ctxT", DT, f"ctxT{dt}") for dt in range(DT)]
        wq_r = w["Wq"].rearrange("(a p) n -> p a n", p=P)
        wk_r = w["Wk"].rearrange("(a p) n -> p a n", p=P)
        for hp in range(DT):  # head pair = output tile of q/k
            qk = {}
            for nm, wr, bcol in (("q", wq_r, bq_col), ("k", wk_r, bk_col)):
                stg = spool([P, DT, P], F32, "wstage", 2, f"w{nm}stg{hp}")
                nc.sync.dma_start(out=stg, in_=wr[:, :, hp * P:(hp + 1) * P])
                blk = spool([P, DT, P], BF16, "wblk", 3, f"w{nm}blk{hp}")
                nc.gpsimd.tensor_copy(out=blk, in_=stg)
                dst = spool([P, L], BF16, "qk", 4, f"{nm}T{hp}")
                for half in range(2):
                    pq = ps1(f"ps{nm}_{hp}_{half}")
                    for kt in range(DT):
                        nc.tensor.matmul(
                            pq[:, 0:512],
                            lhsT=blk[:, kt, :],
                            rhs=x_T[kt][:, half * 512:(half + 1) * 512],
                            start=(kt == 0), stop=(kt == DT - 1))
                    nc.vector.tensor_scalar_add(
                        out=dst[:, half * 512:(half + 1) * 512],
                        in0=pq[:, 0:512],
                        scalar1=bcol[:, hp:hp + 1])
                qk[nm] = dst

            for hl in range(2):  # head within pair
                h = hp * 2 + hl
                r0 = hl * DH
                # scoresT[ki, qi] per ki-tile; exp -> expT bf16
                expT = []
                for kt in range(LT):
                    sc = ps2(f"sc_{h}_{kt}")
                    for half in range(2):
                        nc.tensor.matmul(
                            sc[:, half * 512:(half + 1) * 512],
                            lhsT=qk["k"][r0:r0 + DH, kt * P:(kt + 1) * P],
                            rhs=qk["q"][r0:r0 + DH, half * 512:(half + 1) * 512],
                            start=True, stop=True)
                    et = spool([P, L], BF16, "expT", 5, f"expT_{h}_{kt}")
                    nc.scalar.activation(out=et, in_=sc[:, 0:L], func=AF.Exp,
                                         scale=1.0 / np.sqrt(DH))
                    expT.append(et)
                # ctx^T (+ denominator row 64) ; normalize ; write ctxT
                for half in range(2):
                    pc = ps1(f"psctx_{h}_{half}")
                    for kt in range(LT):
                        nc.tensor.matmul(
                            pc[0:DH + 1, 0:512],
                            lhsT=v_aug[kt][:, h, :],
                            rhs=expT[kt][:, half * 512:(half + 1) * 512],
                            start=(kt == 0), stop=(kt == LT - 1))
                    rs = spool([1, 512], F32, "rsb", 2, f"rs_{h}_{half}")
                    nc.vector.reciprocal(out=rs, in_=pc[DH:DH + 1, 0:512])
                    rb = spool([DH, 512], F32, "rbb", 2, f"rb_{h}_{half}")
                    nc.gpsimd.partition_broadcast(rb, rs)
                    nc.vector.tensor_mul(
                        out=ctxT[hp][r0:r0 + DH,
                                     half * 512:(half + 1) * 512],
                        in0=pc[0:DH, 0:512], in1=rb)

        # ---------------- D: Wo proj + residual + LN1 + transpose ----------
        bo_b = load_bcast(w["bo"], "bo_b")
        g1_b = load_bcast(w["ln1_g"], "g1_b")
        be1_b = load_bcast(w["ln1_b"], "be1_b")
        wo_sb = []
        for kt in range(DT):
            stg = spool([P, D], F32, "wstage", 2, f"wo_stg{kt}")
            nc.sync.dma_start(
                out=stg, in_=w["Wo"].rearrange("(a p) n -> p a n", p=P)[:, kt, :])
            wo = spool([P, D], BF16, "wtile", 8, f"wo_sb{kt}")
            nc.gpsimd.tensor_copy(out=wo, in_=stg)
            wo_sb.append(wo)

        fin_seq = [spool([P, D], BF16, "fin", 16, f"fin_seq{m}")
                   for m in range(LT)]
        stats_pool = ctx.enter_context(tc.tile_pool(name="stats", bufs=4))
        for m in range(LT):
            xr = spool([P, D], F32, "xf32", 2, f"x_re{m}")
            nc.sync.dma_start(out=xr, in_=xg[m * P:(m + 1) * P, :])
            nc.gpsimd.tensor_add(out=xr, in0=xr, in1=bo_b)  # x + bo
            pa = [ps1(f"psat_{m}_0"), ps1(f"psat_{m}_1")]
            for half in range(2):
                for kt in range(DT):
                    nc.tensor.matmul(
                        pa[half][:, 0:512],
                        lhsT=ctxT[kt][:, m * P:(m + 1) * P],
                        rhs=wo_sb[kt][:, half * 512:(half + 1) * 512],
                        start=(kt == 0), stop=(kt == DT - 1))
                nc.vector.tensor_add(out=pa[half][:, 0:512],
                                     in0=pa[half][:, 0:512],
                                     in1=xr[:, half * 512:(half + 1) * 512])
            # LN1 stats straight from PSUM
            st = stats_pool.tile([P, 2, 6], F32, tag="st", bufs=4,
                                 name=f"st1_{m}")
            for half in range(2):
                nc.vector.bn_stats(out=st[:, half, :], in_=pa[half][:, 0:512])
            mv = stats_pool.tile([P, 2], F32, tag="mv", bufs=4, name=f"mv1_{m}")
            nc.vector.bn_aggr(out=mv, in_=st)
            nc.scalar.activation(out=mv[:, 1:2], in_=mv[:, 1:2], func=AF.Sqrt,
                                 bias=eps_t)
            nc.vector.reciprocal(out=mv[:, 1:2], in_=mv[:, 1:2])
            for half in range(2):
                nc.vector.tensor_scalar(
                    out=fin_seq[m][:, half * 512:(half + 1) * 512],
                    in0=pa[half][:, 0:512],
                    scalar1=mv[:, 0:1], scalar2=mv[:, 1:2],
                    op0=ALU.subtract, op1=ALU.mult)
            nc.vector.tensor_mul(out=fin_seq[m], in0=fin_seq[m], in1=g1_b)
            nc.vector.tensor_add(out=fin_seq[m], in0=fin_seq[m], in1=be1_b)

        fin_T = [spool([P, L], BF16, "fin", 16, f"fin_T{j}") for j in range(DT)]
        for m in range(LT):
            for j in range(DT):
                tr = ps2(f"trF_{m}_{j}", dtype=BF16, shape=(P, P))
                nc.tensor.transpose(tr[0:P, 0:P],
                                    fin_seq[m][:, j * P:(j + 1) * P], ident_bf)
                nc.scalar.copy(out=fin_T[j][:, m * P:(m + 1) * P],
                               in_=tr[0:P, 0:P])

        # ---------------- E/F: FFN in two L-halves + LN2 -------------------
        b2_b = load_bcast(w["b2"], "b2_b")
        g2_b = load_bcast(w["ln2_g"], "g2_b")
        be2_b = load_bcast(w["ln2_b"], "be2_b")
        # fin_plus = final + b2 (bf16), used as LN2 residual
        for m in range(LT):
            nc.gpsimd.tensor_add(out=fin_seq[m], in0=fin_seq[m], in1=b2_b)

        w1_r = w["W1"].rearrange("(a p) n -> p a n", p=P)
        w2_r = w["W2"].rearrange("(a p) n -> p a n", p=P)
        for lh in range(2):
            l0 = lh * 512
            # E1: h_T[ht] = relu(W1^T @ final_T + b1) for this L-half
            hT = []
            for ht in range(HT):
                stg = spool([P, DT, P], F32, "wstage", 2, f"w1stg_{lh}_{ht}")
                nc.sync.dma_start(out=stg, in_=w1_r[:, :, ht * P:(ht + 1) * P])
                blk = spool([P, DT, P], BF16, "wblk", 3, f"w1blk_{lh}_{ht}")
                nc.gpsimd.tensor_copy(out=blk, in_=stg)
                ph = ps1(f"psh_{lh}_{ht}")
                for kt in range(DT):
                    nc.tensor.matmul(
                        ph[:, 0:512],
                        lhsT=blk[:, kt, :],
                        rhs=fin_T[kt][:, l0:l0 + 512],
                        start=(kt == 0), stop=(kt == DT - 1))
                h_t = spool([P, 512], BF16, "hT", HT, f"hT_{lh}_{ht}")
                nc.scalar.activation(out=h_t, in_=ph[:, 0:512], func=AF.Relu,
                                     bias=b1_col[:, ht:ht + 1])
                hT.append(h_t)
            # E2: out2 = h @ W2 over all 8 PSUM banks
            o2 = []  # per lt: list of (psum_ap_half0, psum_ap_half1)
            for lt in range(2):
                t = ps2(f"pso2_{lh}_{lt}")
                o2.append([t[:, 0:512], t[:, 512:1024]])
            for lt in range(2, 4):
                o2.append([ps1(f"pso2_{lh}_{lt}_0")[:, 0:512],
                           ps1(f"pso2_{lh}_{lt}_1")[:, 0:512]])
            for ht in range(HT):
                stg = spool([P, D], F32, "wstage", 2, f"w2stg_{lh}_{ht}")
                nc.sync.dma_start(out=stg, in_=w2_r[:, ht, :])
                w2t = spool([P, D], BF16, "wtile", 8, f"w2sb_{lh}_{ht}")
                nc.gpsimd.tensor_copy(out=w2t, in_=stg)
                for lt in range(4):
                    for half in range(2):
                        nc.tensor.matmul(
                            o2[lt][half],
                            lhsT=hT[ht][:, lt * P:(lt + 1) * P],
                            rhs=w2t[:, half * 512:(half + 1) * 512],
                            start=(ht == 0), stop=(ht == HT - 1))
            # F: +final(+b2), LN2, affine, store
            for lt in range(4):
                m = lh * 4 + lt
                for half in range(2):
                    nc.vector.tensor_add(
                        out=o2[lt][half], in0=o2[lt][half],
                        in1=fin_seq[m][:, half * 512:(half + 1) * 512])
                st = stats_pool.tile([P, 2, 6], F32, tag="st", bufs=4,
                                     name=f"st2_{m}")
                for half in range(2):
                    nc.vector.bn_stats(out=st[:, half, :], in_=o2[lt][half])
                mv = stats_pool.tile([P, 2], F32, tag="mv", bufs=4,
                                     name=f"mv2_{m}")
                nc.vector.bn_aggr(out=mv, in_=st)
                nc.scalar.activation(out=mv[:, 1:2], in_=mv[:, 1:2],
                                     func=AF.Sqrt, bias=eps_t)
                nc.vector.reciprocal(out=mv[:, 1:2], in_=mv[:, 1:2])
                o_sb = spool([P, D], F32, "xf32", 2, f"o_sb{m}")
                for half in range(2):
                    nc.vector.tensor_scalar(
                        out=o_sb[:, half * 512:(half + 1) * 512],
                        in0=o2[lt][half],
                        scalar1=mv[:, 0:1], scalar2=mv[:, 1:2],
                        op0=ALU.subtract, op1=ALU.mult)
                nc.vector.tensor_mul(out=o_sb, in0=o_sb, in1=g2_b)
                nc.vector.tensor_add(out=o_sb, in0=o_sb, in1=be2_b)
                nc.sync.dma_start(out=out[m * P:(m + 1) * P, :], in_=o_sb)

    nc.compile()
    return nc


def _get_nc():
    if "nc" not in _CACHE:
        _CACHE["nc"] = _build_nc()
    return _CACHE["nc"]


def kernel(**inputs):
    from concourse.bass_utils import run_bass_kernel_spmd

    nc = _get_nc()
    x = np.ascontiguousarray(np.asarray(inputs["x"], dtype=np.float32))
    weights = {k: np.ascontiguousarray(np.asarray(inputs[k], dtype=np.float32))
               for k in WEIGHT_KEYS}
    in_maps = []
    for g in range(N_CORES):
        b, j = divmod(g, DIL)
        m = {"xg": np.ascontiguousarray(x[b, j::DIL, :])}
        m.update(weights)
        in_maps.append(m)

    res = run_bass_kernel_spmd(nc, in_maps, core_ids=list(range(N_CORES)))
    _CACHE["last_result"] = res

    out = np.empty((B_FULL, S_FULL, D), dtype=np.float32)
    for g, r in enumerate(res.results):
        b, j = divmod(g, DIL)
        out[b, j::DIL, :] = r["out"]
    return out


# revision 17
# speedup vs baseline: 1.6940x; 1.6940x over previous
"""Trainium2 Bass/Tile kernel for a dilated-attention encoder layer.

Sharding: pure data-parallel over the 8 independent (batch, dilation)
attention groups — core g handles rows x[g//2, g%2::2, :].  Each core
gets its own (1024, 1024) activation slice plus a full weight copy and
produces a (1024, 1024) output slice.  No collectives.

Per-core dataflow (L = 1024 positions, D = 1024 features, 16 heads x 64):
  A: load x (f32), PE-transpose -> x_T (bf16 feature-major)
  B: v = x @ Wv, stored seq-major with an all-ones 65th column per head
     (the ones column makes the attention ctx matmul emit the softmax
     denominator as a free extra PSUM row)
  C: per head-pair hp: q_T/k_T = W^T @ x_T; per head: transposed scores
     scoresT[ki,qi] = k q^T (PE row-groups 0-63/64-127 run the two heads
     of a pair concurrently), exp on ACT (f32, no max subtraction —
     |scores|/8 is far below overflow), ctx^T accumulated over ki with
     denominators in row 64; normalize via DVE recip + gpsimd
     partition_broadcast + DVE multiply -> ctxT bf16.
  D: atted = ctx @ Wo; +x +bo and LN1 stats computed in PSUM; affine ->
     final (seq, bf16); PE-transpose -> final_T.
  E: h_T[ht] = relu(W1^T @ final_T + b1) for the full L (W1 read once),
     buffered bf16 (32 tiles).
  F: out2 = h @ W2 in two D-half passes (each W2 element read once);
     full-L x half-D accumulation uses all 8 PSUM banks; first half
     buffered in SBUF f32, then LN2 + affine + store.

All matmuls bf16 (end-to-end rel err ~3e-3 vs the f32 reference); PSUM
accumulation, softmax and LN are f32.  HBM traffic/core: 48MB weights +
x twice + out = 60MB, spread across both HWDGE queues.
"""

import numpy as np

B_FULL, S_FULL, D = 4, 2048, 1024
DIL = 2
L = S_FULL // DIL  # 1024 positions per core
NH, DH = 16, 64
HID = 4096
N_CORES = 8
LN_EPS = 1e-5

WEIGHT_KEYS = [
    "Wq", "bq", "Wk", "bk", "Wv", "bv", "Wo", "bo",
    "ln1_g", "ln1_b", "W1", "b1", "W2", "b2", "ln2_g", "ln2_b",
]

_CACHE = {}


def _build_nc(repeat=1, parts=frozenset({"qkv", "attn", "d", "ffn"})):
    from contextlib import ExitStack

    import concourse.bass as bass
    import concourse.tile as tile
    from concourse import bacc, mybir
    from concourse.masks import make_identity

    F32 = mybir.dt.float32
    BF16 = mybir.dt.bfloat16
    AF = mybir.ActivationFunctionType
    ALU = mybir.AluOpType
    P = 128
    LT = L // P      # 8 L-tiles
    DT = D // P      # 8 D-tiles
    HT = HID // P    # 32 hidden tiles

    nc = bacc.Bacc("TRN2", target_bir_lowering=False, debug=False)

    xg = nc.dram_tensor("xg", [L, D], F32, kind="ExternalInput").ap()
    w = {}
    for k in WEIGHT_KEYS:
        if k == "W1":
            shp = [D, HID]
        elif k == "W2":
            shp = [HID, D]
        elif k.startswith("W"):
            shp = [D, D]
        elif k == "b1":
            shp = [HID]
        else:
            shp = [D]
        w[k] = nc.dram_tensor(k, shp, F32, kind="ExternalInput").ap()
    out = nc.dram_tensor("out", [L, D], F32, kind="ExternalOutput").ap()

    def bcast_ap(vec_ap, parts_=P):
        return bass.AP(
            tensor=vec_ap.tensor, offset=vec_ap.offset,
            ap=[[0, parts_]] + list(vec_ap.ap),
        )

    with tile.TileContext(nc) as tc, ExitStack() as ctx:
        # ---------------- pools ----------------
        sb = ctx.enter_context(tc.tile_pool(name="sb", bufs=2))
        ps = ctx.enter_context(tc.tile_pool(name="ps", bufs=4, space="PSUM"))
        stats_pool = ctx.enter_context(tc.tile_pool(name="stats", bufs=10))
        if repeat > 1:  # timing-only variant: loop the whole body on-device
            ctx.enter_context(tc.For_i(0, repeat, 1))

        def spool(shape, dtype, tag, bufs, name):
            return sb.tile(shape, dtype, tag=tag, bufs=bufs, name=name)

        # one big recycled bf16 scratch class for all activation tensors
        SCRATCH_BUFS = 50

        def scr(name, width=L):
            t = spool([P, NH * (DH + 1)], BF16, "scr", SCRATCH_BUFS, name)
            return t[:, 0:width] if width != NH * (DH + 1) else t

        def ps1(name):
            return ps.tile([P, 512], F32, tag="ps1", bufs=4, name=name)

        def ps2(name, dtype=F32, shape=(P, 1024)):
            return ps.tile(list(shape), dtype, tag="ps2", bufs=2,
                           name=name)

        # alternate weight streams across the two HWDGE queues
        def wq_eng(i):
            return nc.sync if i % 2 == 0 else nc.scalar

        # ---------------- constants ----------------
        ident_f32 = spool([P, P], F32, "idf", 1, "ident_f32")
        make_identity(nc, ident_f32)
        ident_bf = spool([P, P], BF16, "idb", 1, "ident_bf")
        make_identity(nc, ident_bf)
        eps_t = spool([P, 1], F32, "eps", 1, "eps_t")
        nc.vector.memset(eps_t, LN_EPS)

        bq_col = spool([P, DT], F32, "bqc", 1, "bq_col")
        nc.sync.dma_start(out=bq_col, in_=w["bq"].rearrange("(i p) -> p i", p=P))
        bk_col = spool([P, DT], F32, "bkc", 1, "bk_col")
        nc.sync.dma_start(out=bk_col, in_=w["bk"].rearrange("(i p) -> p i", p=P))
        b1_col = spool([P, HT], F32, "b1c", 1, "b1_col")
        nc.sync.dma_start(out=b1_col, in_=w["b1"].rearrange("(i p) -> p i", p=P))

        def load_bcast(vec, name):
            t = spool([P, D], F32, "bcast", 3, name)
            nc.gpsimd.dma_start(out=t, in_=bcast_ap(vec))
            return t

        bv_b = load_bcast(w["bv"], "bv_b")

        # ---------------- A: load x, transpose to x_T (bf16) ----------------
        x_T = [scr(f"x_T{j}") for j in range(DT)]
        for m in range(LT):
            x_seq = spool([P, D], F32, "xf32", 2, f"x_seq{m}")
            nc.sync.dma_start(out=x_seq, in_=xg[m * P:(m + 1) * P, :])
            for j in range(DT):
                tr = ps1(f"trA_{m}_{j}")
                nc.tensor.transpose(tr[0:P, 0:P], x_seq[:, j * P:(j + 1) * P],
                                    ident_f32)
                nc.vector.tensor_copy(out=x_T[j][:, m * P:(m + 1) * P],
                                      in_=tr[0:P, 0:P])

        # ---------------- B: v projection (seq-major, 65-wide per head) ----
        if "qkv" not in parts:  # timing variant: fake v
            v_aug = []
            for m in range(LT):
                va = scr(f"v_aug{m}", width=NH * (DH + 1))
                nc.gpsimd.memset(va, 1.0)
                v_aug.append(va)
        if "attn" not in parts:  # timing variant: fake ctx
            ctxT = []
            for dt in range(DT):
                ct = scr(f"ctxT{dt}")
                nc.gpsimd.memset(ct, 0.5)
                ctxT.append(ct)

        wv_sb = []
        for kt in range(DT if "qkv" in parts else 0):
            stg = spool([P, D], F32, "wstage", 2, f"wv_stg{kt}")
            wq_eng(kt).dma_start(
                out=stg, in_=w["Wv"].rearrange("(a p) n -> p a n", p=P)[:, kt, :])
            wv = spool([P, D], BF16, "wtile", 8, f"wv_sb{kt}")
            nc.gpsimd.tensor_copy(out=wv, in_=stg)
            wv_sb.append(wv)
        if "qkv" in parts:
            v_aug = []
        for m in range(LT if "qkv" in parts else 0):
            va = scr(f"v_aug{m}", width=NH * (DH + 1))
            nc.gpsimd.memset(va[:, :].rearrange("p (h c) -> p h c", c=DH + 1)
                             [:, :, DH:DH + 1], 1.0)
            vav = va.rearrange("p (h c) -> p h c", c=DH + 1)
            for half in range(2):
                pv = ps1(f"psv_{m}_{half}")
                for kt in range(DT):
                    nc.tensor.matmul(
                        pv[:, 0:512],
                        lhsT=x_T[kt][:, m * P:(m + 1) * P],
                        rhs=wv_sb[kt][:, half * 512:(half + 1) * 512],
                        start=(kt == 0), stop=(kt == DT - 1))
                nc.vector.tensor_add(
                    out=vav[:, half * 8:(half + 1) * 8, 0:DH],
                    in0=pv.rearrange("p (h c) -> p h c", c=DH),
                    in1=bv_b[:, half * 512:(half + 1) * 512].rearrange(
                        "p (h c) -> p h c", c=DH))
            v_aug.append(va)

        # ---------------- C: per head-pair q/k projection + attention ------
        if "attn" in parts:
            ctxT = [scr(f"ctxT{dt}") for dt in range(DT)]
        wq_r = w["Wq"].rearrange("(a p) n -> p a n", p=P)
        wk_r = w["Wk"].rearrange("(a p) n -> p a n", p=P)
        for hp in range(DT if "qkv" in parts else 0):  # head pair
            qk = {}
            for wi, (nm, wr, bcol) in enumerate(
                    (("q", wq_r, bq_col), ("k", wk_r, bk_col))):
                stg = spool([P, DT, P], F32, "wstage", 2, f"w{nm}stg{hp}")
                wq_eng(hp * 2 + wi).dma_start(
                    out=stg, in_=wr[:, :, hp * P:(hp + 1) * P])
                blk = spool([P, DT, P], BF16, "wblk", 3, f"w{nm}blk{hp}")
                nc.gpsimd.tensor_copy(out=blk, in_=stg)
                dst = scr(f"{nm}T{hp}")
                for half in range(2):
                    pq = ps1(f"ps{nm}_{hp}_{half}")
                    for kt in range(DT):
                        nc.tensor.matmul(
                            pq[:, 0:512],
                            lhsT=blk[:, kt, :],
                            rhs=x_T[kt][:, half * 512:(half + 1) * 512],
                            start=(kt == 0), stop=(kt == DT - 1))
                    nc.vector.tensor_scalar_add(
                        out=dst[:, half * 512:(half + 1) * 512],
                        in0=pq[:, 0:512],
                        scalar1=bcol[:, hp:hp + 1])
                qk[nm] = dst

            if "attn" not in parts:
                continue
            # scoresT per ki-tile, both heads of the pair interleaved so the
            # two matmuls land in disjoint PE row-groups (0-63 / 64-127) and
            # run concurrently.
            expT = {0: [], 1: []}
            for kt in range(LT):
                sc = {}
                for hl in range(2):
                    r0 = hl * DH
                    s = ps2(f"sc_{hp}_{hl}_{kt}")
                    for half in range(2):
                        nc.tensor.matmul(
                            s[:, half * 512:(half + 1) * 512],
                            lhsT=qk["k"][r0:r0 + DH, kt * P:(kt + 1) * P],
                            rhs=qk["q"][r0:r0 + DH,
                                        half * 512:(half + 1) * 512],
                            start=True, stop=True)
                    sc[hl] = s
                for hl in range(2):
                    et = scr(f"expT_{hp}_{hl}_{kt}")
                    nc.scalar.activation(out=et, in_=sc[hl][:, 0:L],
                                         func=AF.Exp, scale=1.0 / np.sqrt(DH))
                    expT[hl].append(et)
            for hl in range(2):
                h = hp * 2 + hl
                for half in range(2):
                    pc = ps1(f"psctx_{h}_{half}")
                    for kt in range(LT):
                        nc.tensor.matmul(
                            pc[0:DH + 1, 0:512],
                            lhsT=v_aug[kt].rearrange(
                                "p (h c) -> p h c", c=DH + 1)[:, h, :],
                            rhs=expT[hl][kt][:, half * 512:(half + 1) * 512],
                            start=(kt == 0), stop=(kt == LT - 1))
                    rs = spool([1, 512], F32, "rsb", 2, f"rs_{h}_{half}")
                    nc.vector.reciprocal(out=rs, in_=pc[DH:DH + 1, 0:512])
                    rb = spool([DH, 512], F32, "rbb", 2, f"rb_{h}_{half}")
                    nc.gpsimd.partition_broadcast(rb, rs)
                    nc.vector.tensor_mul(
                        out=ctxT[hp][hl * DH:hl * DH + DH,
                                     half * 512:(half + 1) * 512],
                        in0=pc[0:DH, 0:512], in1=rb)

        # ---------------- D: Wo proj + residual + LN1 + transpose ----------
        bo_b = load_bcast(w["bo"], "bo_b")
        g1_b = load_bcast(w["ln1_g"], "g1_b")
        be1_b = load_bcast(w["ln1_b"], "be1_b")
        wo_sb = []
        for kt in range(DT if "d" in parts else 0):
            stg = spool([P, D], F32, "wstage", 2, f"wo_stg{kt}")
            wq_eng(kt).dma_start(
                out=stg, in_=w["Wo"].rearrange("(a p) n -> p a n", p=P)[:, kt, :])
            wo = spool([P, D], BF16, "wtile", 8, f"wo_sb{kt}")
            nc.gpsimd.tensor_copy(out=wo, in_=stg)
            wo_sb.append(wo)

        fin_seq = [scr(f"fin_seq{m}") for m in range(LT)]
        if "d" not in parts:  # timing variant: final = x
            for m in range(LT):
                xr = spool([P, D], F32, "xf32", 2, f"x_re{m}")
                nc.sync.dma_start(out=xr, in_=xg[m * P:(m + 1) * P, :])
                nc.vector.tensor_copy(out=fin_seq[m], in_=xr)
        for m in range(LT if "d" in parts else 0):
            xr = spool([P, D], F32, "xf32", 2, f"x_re{m}")
            nc.sync.dma_start(out=xr, in_=xg[m * P:(m + 1) * P, :])
            nc.gpsimd.tensor_add(out=xr, in0=xr, in1=bo_b)  # x + bo
            pa = [ps1(f"psat_{m}_0"), ps1(f"psat_{m}_1")]
            for half in range(2):
                for kt in range(DT):
                    nc.tensor.matmul(
                        pa[half][:, 0:512],
                        lhsT=ctxT[kt][:, m * P:(m + 1) * P],
                        rhs=wo_sb[kt][:, half * 512:(half + 1) * 512],
                        start=(kt == 0), stop=(kt == DT - 1))
                nc.vector.tensor_add(out=pa[half][:, 0:512],
                                     in0=pa[half][:, 0:512],
                                     in1=xr[:, half * 512:(half + 1) * 512])
            st = stats_pool.tile([P, 2, 6], F32, tag="st", bufs=4,
                                 name=f"st1_{m}")
            for half in range(2):
                nc.vector.bn_stats(out=st[:, half, :], in_=pa[half][:, 0:512])
            mv = stats_pool.tile([P, 2], F32, tag="mv", bufs=4, name=f"mv1_{m}")
            nc.vector.bn_aggr(out=mv, in_=st)
            nc.scalar.activation(out=mv[:, 1:2], in_=mv[:, 1:2], func=AF.Sqrt,
                                 bias=eps_t)
            nc.vector.reciprocal(out=mv[:, 1:2], in_=mv[:, 1:2])
            for half in range(2):
                nc.vector.tensor_scalar(
                    out=fin_seq[m][:, half * 512:(half + 1) * 512],
                    in0=pa[half][:, 0:512],
                    scalar1=mv[:, 0:1], scalar2=mv[:, 1:2],
                    op0=ALU.subtract, op1=ALU.mult)
            nc.vector.tensor_mul(out=fin_seq[m], in0=fin_seq[m], in1=g1_b)
            nc.vector.tensor_add(out=fin_seq[m], in0=fin_seq[m], in1=be1_b)

        # ---------------- E: h_T = relu(W1^T @ final_T + b1), full L -------
        b2_b = load_bcast(w["b2"], "b2_b")
        g2_b = load_bcast(w["ln2_g"], "g2_b")
        be2_b = load_bcast(w["ln2_b"], "be2_b")

        if "ffn" not in parts:  # timing variant: out = final
            for m in range(LT):
                o_sb = spool([P, D], F32, "xf32", 2, f"o_sb{m}")
                nc.vector.tensor_copy(out=o_sb, in_=fin_seq[m])
                nc.sync.dma_start(out=out[m * P:(m + 1) * P, :], in_=o_sb)

        ffn = "ffn" in parts
        fin_T = [scr(f"fin_T{j}") for j in range(DT if ffn else 0)]
        for m in range(LT if ffn else 0):
            for j in range(DT):
                tr = ps2(f"trF_{m}_{j}", dtype=BF16, shape=(P, P))
                nc.tensor.transpose(tr[0:P, 0:P],
                                    fin_seq[m][:, j * P:(j + 1) * P], ident_bf)
                nc.scalar.copy(out=fin_T[j][:, m * P:(m + 1) * P],
                               in_=tr[0:P, 0:P])
        # fin_plus = final + b2 (bf16): LN2 residual
        for m in range(LT if ffn else 0):
            nc.gpsimd.tensor_add(out=fin_seq[m], in0=fin_seq[m], in1=b2_b)

        w1_r = w["W1"].rearrange("(a p) n -> p a n", p=P)
        hT = []
        for ht in range(HT if ffn else 0):
            stg = spool([P, DT, P], F32, "wstage", 2, f"w1stg_{ht}")
            wq_eng(ht).dma_start(out=stg, in_=w1_r[:, :, ht * P:(ht + 1) * P])
            blk = spool([P, DT, P], BF16, "wblk", 3, f"w1blk_{ht}")
            nc.gpsimd.tensor_copy(out=blk, in_=stg)
            ph = ps2(f"psh_{ht}")
            for half in range(2):
                for kt in range(DT):
                    nc.tensor.matmul(
                        ph[:, half * 512:(half + 1) * 512],
                        lhsT=blk[:, kt, :],
                        rhs=fin_T[kt][:, half * 512:(half + 1) * 512],
                        start=(kt == 0), stop=(kt == DT - 1))
            h_t = scr(f"hT_{ht}")
            nc.scalar.activation(out=h_t, in_=ph[:, 0:L], func=AF.Relu,
                                 bias=b1_col[:, ht:ht + 1])
            hT.append(h_t)

        # ---------------- F: out2 = h @ W2 in two D-half passes + LN2 ------
        o2buf = [spool([P, 512], F32, "o2buf", LT, f"o2buf{m}")
                 for m in range(LT if ffn else 0)]
        sts = [stats_pool.tile([P, 2, 6], F32, tag="st2", bufs=LT,
                               name=f"st2_{m}") for m in range(LT if ffn else 0)]
        for dh in range(2 if ffn else 0):
            # lt 0..3 use the two ps2 slots (2 halves each); lt 4..7 use ps1
            o2 = []
            for i in range(2):
                t = ps2(f"pso2_{dh}_{i}")
                o2 += [t[:, 0:512], t[:, 512:1024]]
            o2 += [ps1(f"pso2_{dh}_{lt}")[:, 0:512] for lt in range(4, 8)]
            for ht in range(HT):
                stg = spool([P, 512], F32, "wstage", 2, f"w2stg_{dh}_{ht}")
                wq_eng(ht).dma_start(
                    out=stg,
                    in_=w["W2"][ht * P:(ht + 1) * P,
                                dh * 512:(dh + 1) * 512])
                w2t = spool([P, 512], BF16, "wtile", 8, f"w2sb_{dh}_{ht}")
                nc.gpsimd.tensor_copy(out=w2t, in_=stg)
                for lt in range(8):
                    nc.tensor.matmul(
                        o2[lt],
                        lhsT=hT[ht][:, lt * P:(lt + 1) * P],
                        rhs=w2t,
                        start=(ht == 0), stop=(ht == HT - 1))
            for m in range(8):
                # +final(+b2) residual for this half
                nc.vector.tensor_add(
                    out=o2[m], in0=o2[m],
                    in1=fin_seq[m][:, dh * 512:(dh + 1) * 512])
                nc.vector.bn_stats(out=sts[m][:, dh, :], in_=o2[m])
                if dh == 0:
                    nc.vector.tensor_copy(out=o2buf[m], in_=o2[m])
                else:
                    mv = stats_pool.tile([P, 2], F32, tag="mv2", bufs=4,
                                         name=f"mv2_{m}")
                    nc.vector.bn_aggr(out=mv, in_=sts[m])
                    nc.scalar.activation(out=mv[:, 1:2], in_=mv[:, 1:2],
                                         func=AF.Sqrt, bias=eps_t)
                    nc.vector.reciprocal(out=mv[:, 1:2], in_=mv[:, 1:2])
                    o_sb = spool([P, D], F32, "xf32", 2, f"o_sb{m}")
                    nc.vector.tensor_scalar(
                        out=o_sb[:, 0:512], in0=o2buf[m],
                        scalar1=mv[:, 0:1], scalar2=mv[:, 1:2],
                        op0=ALU.subtract, op1=ALU.mult)
                    nc.vector.tensor_scalar(
                        out=o_sb[:, 512:1024], in0=o2[m],
                        scalar1=mv[:, 0:1], scalar2=mv[:, 1:2],
                        op0=ALU.subtract, op1=ALU.mult)
                    nc.vector.tensor_mul(out=o_sb, in0=o_sb, in1=g2_b)
                    nc.vector.tensor_add(out=o_sb, in0=o_sb, in1=be2_b)
                    nc.sync.dma_start(out=out[m * P:(m + 1) * P, :], in_=o_sb)

    nc.compile()
    return nc


def _get_nc():
    if "nc" not in _CACHE:
        _CACHE["nc"] = _build_nc()
    return _CACHE["nc"]


def kernel(**inputs):
    from concourse.bass_utils import run_bass_kernel_spmd

    nc = _get_nc()
    x = np.ascontiguousarray(np.asarray(inputs["x"], dtype=np.float32))
    weights = {k: np.ascontiguousarray(np.asarray(inputs[k], dtype=np.float32))
               for k in WEIGHT_KEYS}
    in_maps = []
    for g in range(N_CORES):
        b, j = divmod(g, DIL)
        m = {"xg": np.ascontiguousarray(x[b, j::DIL, :])}
        m.update(weights)
        in_maps.append(m)

    res = run_bass_kernel_spmd(nc, in_maps, core_ids=list(range(N_CORES)))
    _CACHE["last_result"] = res

    out = np.empty((B_FULL, S_FULL, D), dtype=np.float32)
    for g, r in enumerate(res.results):
        b, j = divmod(g, DIL)
        out[b, j::DIL, :] = r["out"]
    return out
